# revision 52
# baseline (speedup 1.0000x reference)
"""Multi-head attention Trainium2 kernel, 8-way sharded, mask-compacted.

Problem: x[4,2048,1024] -> qkv proj (w_qkv [3072,1024]) -> 16-head attention
with key-padding mask -> tail proj (w_tail [1024,1024]) + b_tail.

Sharding: 8 shards = 4 batches x 2 head-groups (8 heads each). Each core
computes, for its (batch b, head-group hg):
  - q projection of x[b] for its 8 heads (all 2048 tokens)
  - k/v projections of the MASK-KEPT tokens only (host-compacted; masked
    keys contribute exactly zero to the softmax, so dropping them up front
    is exact and halves the attention work)
  - [kept x 2048] masked attention per head
  - partial tail matmul y_part = attn_cat @ w_tail[:, cat_slice].T
Host unshards: out[b] = y_part[2b] + y_part[2b+1] + b_tail.  No collectives.

All matmul operands are bf16 (PE 1 cyc/row); PSUM accumulation is f32.

Attention runs in (pair, 512-q-column-quarter) units: per key block both
heads' K=64 score matmuls write different column halves (different PSUM
banks) of one [128,1024] stp tile at row tiles (0,0)/(64,0), so they share
one WAR dep (the single exp that reads the whole tile) and execute
CONCURRENTLY in the PE array.  One ACT exp per key block covers both heads.
Next-pair k/q projection chunks and (in the last pair) ready tail blocks
fill the PE between score/AV groups.

Softmax denominator comes from a ones-column appended to V.  The per-token
reciprocal chain (DMA-scatter to 16 lanes -> DVE recip -> gather to
partitions 0/32 -> DVE stream_shuffle broadcast -> gpsimd multiply) is
software-pipelined two units deep so no engine FIFO ever blocks on a DMA;
a plain broadcast DMA explodes into 64x4KB packets and drowns the queue.
PE warm-up matmuls on memset scratch hold the HAM clock gate at K=8/8
through the DMA preamble.
"""

import time as _time

import numpy as np
from contextlib import ExitStack

import concourse.bass as bass
import concourse.mybir as mybir
import concourse.tile as tile
from concourse.bass_utils import run_bass_kernel_spmd

# ---------------------------------------------------------------------------
# walrus in this env accepts at most 2 sync waits per instruction; Tile's
# scheduler emits up to 10. Post-pass: peel excess waits onto same-engine
# NoOps inserted immediately before the offending instruction (same engine
# stream position => identical synchronization semantics).
MAX_WAITS = 1


def split_excess_waits(nc):
    for fn in nc.m.functions:
        for bb in fn.blocks:
            insts = list(bb.instructions)
            out = []
            changed = False
            for inst in insts:
                si = inst.sync_info
                waits = list(si.on_wait) if si is not None else []
                if len(waits) > MAX_WAITS:
                    extra = waits[:-MAX_WAITS]
                    for ci in range(0, len(extra), MAX_WAITS):
                        chunk = extra[ci:ci + MAX_WAITS]
                        nop = mybir.InstNoOp(
                            name=f"{inst.name}-ws{ci}", ins=[], outs=[])
                        nop.engine = inst.engine
                        nop.sync_info = mybir.SyncInfo(
                            on_wait=chunk, on_update=[])
                        out.append(nop)
                    inst.sync_info = mybir.SyncInfo(
                        on_wait=waits[-MAX_WAITS:],
                        on_update=list(si.on_update))
                    changed = True
                out.append(inst)
            if changed:
                bb.instructions = out
# ---------------------------------------------------------------------------

D_MODEL = 1024
N_HEAD = 16
D_HEAD = 64
BN, T = 4, 2048
HPC = 8                      # heads per core
NPAIR = HPC // 2             # head pairs (2 heads share a 128-row tile)
CAT = HPC * D_HEAD           # 512 per-core tail contraction
QH = T // 2                  # q processed in two halves of 1024
KC = D_MODEL // 128          # 8 contraction chunks
F32 = mybir.dt.float32
BF16 = mybir.dt.bfloat16
LAG = 4                      # ST->AV software pipeline depth (key blocks)
NEG = -30000.0               # additive bias for padded keys: exp -> 0


def build_nc(tkp, split_waits=True):
    """tkp: padded kept-key count (multiple of 128)."""
    nkbk = tkp // 128        # kept-key blocks
    nc = bass.Bass()
    xT = nc.declare_dram_parameter("xT", [D_MODEL, T], BF16, isOutput=False)
    xkT = nc.declare_dram_parameter("xkT", [D_MODEL, tkp], BF16, isOutput=False)
    wqT = nc.declare_dram_parameter("wqT", [D_MODEL, CAT], BF16, isOutput=False)
    wkT = nc.declare_dram_parameter("wkT", [D_MODEL, CAT], BF16, isOutput=False)
    wvT = nc.declare_dram_parameter("wvT", [D_MODEL, CAT], BF16, isOutput=False)
    wtailT = nc.declare_dram_parameter("wtailT", [CAT, D_MODEL], BF16, isOutput=False)
    maskb_d = nc.declare_dram_parameter("maskb", [tkp], F32, isOutput=False)
    ones8 = nc.declare_dram_parameter("ones8", [128, HPC], BF16, isOutput=False)
    y = nc.declare_dram_parameter("y", [T, D_MODEL], BF16, isOutput=True)

    with ExitStack() as ctx:
        tc = ctx.enter_context(tile.TileContext(nc))

        # ---- persistent pools
        const = ctx.enter_context(tc.tile_pool(name="const", bufs=1))
        wpool = ctx.enter_context(tc.tile_pool(name="w", bufs=1))
        xpool = ctx.enter_context(tc.tile_pool(name="x", bufs=1))
        qk_pool = ctx.enter_context(tc.tile_pool(name="qk", bufs=1))
        vaug_pool = ctx.enter_context(tc.tile_pool(name="vaug", bufs=1))
        num_pool = ctx.enter_context(tc.tile_pool(name="num", bufs=1))

        onesb = const.tile([128, HPC], BF16)
        nc.scalar.dma_start(out=onesb, in_=ones8[:, :])
        maskb = const.tile([128, nkbk], F32)
        nc.scalar.dma_start(
            out=maskb, in_=maskb_d.rearrange("(j p) -> p j", p=128))
        warm = const.tile([1, 1], F32)
        nc.scalar.copy(out=warm, in_=maskb[0:1, 0:1])  # preload ACT table
        # scratch operands for PE warm-up matmuls during the DMA preamble;
        # memset (engine op, no DMA) so the warm-ups start at ~0.5us, well
        # before the DMA queues spin up (~7us)
        wuscr = const.tile([128, 512], BF16)
        nc.gpsimd.memset(wuscr, 0.0)
        wub = const.tile([128, HPC], BF16)
        nc.gpsimd.memset(wub, 0.0)

        # weights resident in SBUF.  All DMAs are split per 128-row chunk so
        # consumers unblock as soon as their chunk lands, and ordered by
        # first use: xk + wv/wk (V & K proj) first, wtail last.
        wq_sb = wpool.tile([128, KC, CAT], BF16)
        wk_sb = wpool.tile([128, KC, CAT], BF16)
        wv_sb = wpool.tile([128, KC, CAT], BF16)
        wt_sb = wpool.tile([128, CAT // 128, D_MODEL], BF16)
        xkts = [xpool.tile([128, tkp], BF16, tag=f"xk{kc}", name=f"xk{kc}")
                for kc in range(KC)]
        xts = [xpool.tile([128, T], BF16, tag=f"x{kc}", name=f"x{kc}")
               for kc in range(KC)]
        # sync queue: xk (V/K proj inputs), then x half 0, then wtail
        for kc in range(KC):
            nc.sync.dma_start(out=xkts[kc],
                              in_=xkT[kc * 128:(kc + 1) * 128, :])
        for kc in range(KC):
            nc.sync.dma_start(
                out=xts[kc][:, 0:QH],
                in_=xT[kc * 128:(kc + 1) * 128, 0:QH])
        nc.sync.dma_start(out=wt_sb, in_=wtailT.rearrange("(c p) o -> p c o", p=128))
        # gpsimd software-DGE queue: weights per chunk, then x half 1
        for kc in range(KC):
            nc.gpsimd.dma_start(out=wv_sb[:, kc, :],
                                in_=wvT[kc * 128:(kc + 1) * 128, :])
        for kc in range(KC):
            nc.gpsimd.dma_start(out=wk_sb[:, kc, :],
                                in_=wkT[kc * 128:(kc + 1) * 128, :])
        for kc in range(KC):
            nc.gpsimd.dma_start(out=wq_sb[:, kc, :],
                                in_=wqT[kc * 128:(kc + 1) * 128, :])
        for kc in range(KC):
            nc.gpsimd.dma_start(
                out=xts[kc][:, QH:T],
                in_=xT[kc * 128:(kc + 1) * 128, QH:T])

        # persistent intermediates
        qts = [qk_pool.tile([128, T], BF16, tag=f"qt{j}", name=f"qt{j}")
               for j in range(NPAIR)]
        kts = [qk_pool.tile([128, tkp], BF16, tag=f"kt{j}", name=f"kt{j}")
               for j in range(NPAIR)]
        vaugs = [vaug_pool.tile([128, HPC, D_HEAD + 1], BF16, tag=f"va{t}",
                                name=f"va{t}") for t in range(nkbk)]
        nums = [num_pool.tile([128, T], BF16, tag=f"nm{j}", name=f"nm{j}")
                for j in range(NPAIR)]

        # ---- phase 1a: V projection (kept tokens), vps has all 8 PSUM banks
        with tc.tile_pool(name="vps", bufs=1, space="PSUM") as vps:
            # PE warm-up: dense dummy matmuls while the input DMAs land, so
            # the HAM clock-gate releases (K=8/8) before real work starts.
            # Reuses the vp0 slot (WAW: V proj's first group serializes after,
            # which is the PE program order anyway).
            wups = vps.tile([128, CAT], F32, tag="vp0", name="wu")
            for _ in range(14):
                nc.tensor.matmul(wups[0:HPC, :], wub, wuscr,
                                 start=True, stop=True, skip_group_check=True)
            done = 0
            while done < nkbk:
                g = min(8, nkbk - done)
                vp = [vps.tile([128, CAT], F32, tag=f"vp{i}", name=f"vp{i}")
                      for i in range(g)]
                for kc in range(KC):
                    for i in range(g):
                        nc.tensor.matmul(
                            vp[i],
                            xkts[kc][:, (done + i) * 128:(done + i + 1) * 128],
                            wv_sb[:, kc, :],
                            start=(kc == 0), stop=(kc == KC - 1),
                        )
                for i in range(g):
                    va = vaugs[done + i]
                    nc.gpsimd.memset(va[:, :, D_HEAD:D_HEAD + 1], 1.0)
                    nc.scalar.copy(
                        out=va[:, :, 0:D_HEAD],
                        in_=vp[i].rearrange("p (h d) -> p h d", h=HPC),
                    )
                done += g

        # ---- k/q projection helpers (pps pool: [128,512] x2 = 2 PSUM banks)
        KCH = [(c * 512, min(512, tkp - c * 512)) for c in range((tkp + 511) // 512)]

        def _copy(eng, out, in_):
            if eng is nc.scalar:
                eng.copy(out=out, in_=in_)
            else:
                eng.tensor_copy(out=out, in_=in_)

        def emit_kproj_chunk(pps, j, c0, csz, eng):
            pp = pps.tile([128, QH], F32, tag="stp", name="pp")
            for kc in range(KC):
                nc.tensor.matmul(
                    pp[:, 0:csz],
                    wk_sb[:, kc, j * 128:(j + 1) * 128],
                    xkts[kc][:, c0:c0 + csz],
                    start=(kc == 0), stop=(kc == KC - 1),
                )
            _copy(eng, kts[j][:, c0:c0 + csz], pp[:, 0:csz])

        def emit_qproj_chunk(pps, j, c, eng):
            pp = pps.tile([128, QH], F32, tag="stp", name="pp")
            for kc in range(KC):
                nc.tensor.matmul(
                    pp[:, 0:512],
                    wq_sb[:, kc, j * 128:(j + 1) * 128],
                    xts[kc][:, c * 512:(c + 1) * 512],
                    start=(kc == 0), stop=(kc == KC - 1),
                )
            _copy(eng, qts[j][:, c * 512:(c + 1) * 512], pp[:, 0:512])

        def proj_chunks_for_pair(j, eng):
            # k first (attention consumes k of every block before q half 2)
            for c0, csz in KCH:
                yield lambda pps, j=j, c0=c0, csz=csz, eng=eng: \
                    emit_kproj_chunk(pps, j, c0, csz, eng)
            for c in range(T // 512):
                yield lambda pps, j=j, c=c, eng=eng: \
                    emit_qproj_chunk(pps, j, c, eng)

# ---- phase 2: attention.  Units are (pair, quarter-of-512-q-columns):
        # per key block BOTH heads' K=64 score matmuls write different
        # column halves (= different PSUM banks) of ONE [128,1024] stp tile
        # at row tiles (0,0)/(64,0).  They share a single WAR dependency
        # (the one exp that reads the whole tile), so the Tile scheduler
        # keeps them adjacent and they execute CONCURRENTLY in the array —
        # the score phase costs ~1x instead of 2x.  One exp per key block
        # covers both heads; ACT paces the loop and projection chunks for
        # the next pair fill the PE slack.
        QQ = 512                 # q-columns per unit
        with tc.tile_pool(name="p_sb", bufs=4) as p_pool, \
             tc.tile_pool(name="r_sb", bufs=4) as r_pool, \
             tc.tile_pool(name="bc_sb", bufs=3) as bc_pool, \
             tc.tile_pool(name="av_sb", bufs=7) as av_pool, \
             tc.tile_pool(name="y_sb", bufs=3) as y_pool, \
             tc.tile_pool(name="stps", bufs=3, space="PSUM") as stps, \
             tc.tile_pool(name="avps", bufs=2, space="PSUM") as avps:
            pps = stps

            def emit_tail_tb(tb, eng, store_q=None, split=False):
                yp = stps.tile([128, D_MODEL], F32, tag="stp", name="yp")
                # c outer: the stationary operand (nums[c] slice) is shared
                # by the two n-slab matmuls — one weight load per c
                for c in range(CAT // 128):
                    for n in range(D_MODEL // 512):
                        nc.tensor.matmul(
                            yp[:, n * 512:(n + 1) * 512],
                            nums[c][:, tb * 128:(tb + 1) * 128],
                            wt_sb[:, c, n * 512:(n + 1) * 512],
                            start=(c == 0), stop=(c == CAT // 128 - 1),
                        )
                y_sb = y_pool.tile([128, D_MODEL], BF16, tag="ys")
                if split:
                    # last blocks: evacuate the two yp banks on BOTH engines
                    # and store the halves on both queues, in parallel —
                    # halves the serial copy+store chain the final pool
                    # barrier waits on
                    nc.vector.tensor_copy(out=y_sb[:, 0:512],
                                          in_=yp[:, 0:512])
                    nc.scalar.copy(out=y_sb[:, 512:1024],
                                   in_=yp[:, 512:1024])
                    nc.sync.dma_start(
                        out=y[tb * 128:(tb + 1) * 128, 0:512],
                        in_=y_sb[:, 0:512])
                    nc.gpsimd.dma_start(
                        out=y[tb * 128:(tb + 1) * 128, 512:1024],
                        in_=y_sb[:, 512:1024])
                    return
                _copy(eng, y_sb, yp)
                # bf16 store; mid-attention fills ride the gpsimd queue so
                # sync stays free for the norm-chain hops
                (store_q or nc.gpsimd).dma_start(
                    out=y[tb * 128:(tb + 1) * 128, :], in_=y_sb)

            # pair-0 k/q projection up front (ACT idle: use it for copies)
            for emit in proj_chunks_for_pair(0, nc.scalar):
                emit(pps)

            def fat_keeper(ptile):
                # ~0.2us dummy matmul: fills PE when no proj work remains
                # (pair 3) so the HAM clock-gate stays at full speed
                nc.tensor.matmul(
                    ptile[0:HPC, 0:512], onesb, nums[0][:, 0:512],
                    start=True, stop=True, skip_group_check=True)

            # Per-token reciprocal of the ones-row, staged so that no DVE
            # instruction ever sits in the (strict-FIFO) vector queue waiting
            # on a DMA.  The denominator row is DMA-scattered over 16 lanes
            # (16 small packets — a full 64-partition broadcast DMA explodes
            # into 64x4KB packets at ~2.3us latency each and drowns the
            # hardware queue), recip'd on DVE, gathered back to partitions
            # 0 and 32, then broadcast across partitions with a DVE
            # stream_shuffle (mask=0: every lane of each 32-group reads
            # lane 0).  The final multiply runs on the idle gpsimd engine.
            def norm_start(av_sb, pair, r0, q0):
                d16 = r_pool.tile([16, QQ // 16], F32, tag="d64",
                                  name="d16")
                nc.sync.dma_start(
                    out=d16, in_=av_sb[D_HEAD:D_HEAD + 1, :])
                return [av_sb, pair, r0, q0, d16, None, 0]

            def norm_step1(st):
                if st[6] >= 1:
                    return
                av_sb, pair, r0, q0, d16, _, _ = st
                r16 = r_pool.tile([16, QQ // 16], F32, tag="r64",
                                  name="r16")
                nc.vector.reciprocal(out=r16, in_=d16)
                r2 = bc_pool.tile([D_HEAD, QQ], F32, tag="r2", name="r2")
                nc.sync.dma_start(out=r2[0:1, :], in_=r16)
                nc.sync.dma_start(out=r2[32:33, :], in_=r16)
                st[5] = r2
                st[6] = 1

            def norm_step2(st):
                norm_step1(st)
                av_sb, pair, r0, q0, d16, r2, _ = st
                bc_sb = bc_pool.tile([D_HEAD, QQ], F32, tag="bc",
                                     name="bc_sb")
                nc.vector.stream_shuffle(out=bc_sb, in_=r2, mask=[0] * 32)
                nc.gpsimd.tensor_tensor(
                    nums[pair][r0:r0 + 64, q0:q0 + QQ],
                    av_sb[0:D_HEAD, :], bc_sb,
                    mybir.AluOpType.mult,
                )

            LAG2 = 2  # ST->exp->AV pipeline depth (key blocks)
            pending_groups = []
            tail_emitted = 0
            for pair in range(NPAIR):
                proj_iter = iter(
                    proj_chunks_for_pair(pair + 1, nc.vector)
                    if pair + 1 < NPAIR else [])
                hA, hB = 2 * pair, 2 * pair + 1
                ktA = kts[pair][0:64, :]
                ktB = kts[pair][64:128, :]
                qtA = qts[pair][0:64, :]
                qtB = qts[pair][64:128, :]
                last = pair == NPAIR - 1
                for qtr in range(T // QQ):
                    q0 = qtr * QQ
                    avpA = avps.tile([D_HEAD + 1, QQ], F32, tag="avp",
                                     name="avpA")
                    avpB = avps.tile([D_HEAD + 1, QQ], F32, tag="avp",
                                     name="avpB")
                    p_tiles = {}

                    def emit_st_exp(kb):
                        # both heads' scores into different column halves
                        # (= different banks) of one stp tile: ST_A rows
                        # 0-63 / cols 0-511, ST_B rows 64-127 / cols
                        # 512-1023 — adjacent row-tiled matmuls, overlap
                        stp = stps.tile([128, 2 * QQ], F32, tag="stp",
                                        name="stp")
                        nc.tensor.matmul(
                            stp[:, 0:QQ],
                            ktA[:, kb * 128:(kb + 1) * 128],
                            qtA[:, q0:q0 + QQ],
                            start=True, stop=True,
                        )
                        nc.tensor.matmul(
                            stp[:, QQ:2 * QQ],
                            ktB[:, kb * 128:(kb + 1) * 128],
                            qtB[:, q0:q0 + QQ],
                            start=True, stop=True,
                        )
                        # one exp covers both heads' halves
                        p_sb = p_pool.tile([128, 2 * QQ], BF16, tag="p",
                                           name="p_sb")
                        nc.scalar.activation(
                            out=p_sb, in_=stp,
                            func=mybir.ActivationFunctionType.Exp,
                            bias=maskb[:, kb:kb + 1], scale=0.125,
                        )
                        p_tiles[kb] = p_sb

                    def emit_av(kb):
                        p_sb = p_tiles.pop(kb)
                        nc.tensor.matmul(
                            avpA, vaugs[kb][:, hA, :], p_sb[:, 0:QQ],
                            start=(kb == 0), stop=(kb == nkbk - 1),
                        )
                        nc.tensor.matmul(
                            avpB, vaugs[kb][:, hB, :], p_sb[:, QQ:2 * QQ],
                            start=(kb == 0), stop=(kb == nkbk - 1),
                        )

                    # filler budget: next-pair proj chunks (7 over 4 units);
                    # in the last pair, tail blocks whose q-columns are
                    # already fully normalized serve as fill instead
                    budget = (2 if qtr < 3 else 1) if not last else 3
                    for kb in range(nkbk):
                        if kb % 3 == 2 and budget > 0:
                            emit = next(proj_iter, None)
                            if emit is not None:
                                emit(pps)
                            elif last and qtr >= 2 and tail_emitted < 4 * (qtr - 1):
                                emit_tail_tb(tail_emitted, nc.vector)
                                tail_emitted += 1
                            budget -= 1
                        emit_st_exp(kb)
                        if kb >= LAG2:
                            emit_av(kb - LAG2)
                    for kb in range(max(0, nkbk - LAG2), nkbk):
                        emit_av(kb)
                    while budget > 0:
                        emit = next(proj_iter, None)
                        if emit is None:
                            break
                        emit(pps)
                        budget -= 1
                    # Evacuate both PSUM accumulators first (so the next
                    # unit's AVs never wait), then advance the norm chains:
                    # start(u) -> recip(u-1) -> multiply(u-2 mid / u-1 last
                    # pair); each stage's DMA inputs landed a unit ago, so
                    # the DVE FIFO never blocks on them.
                    avA = av_pool.tile([D_HEAD + 1, QQ], F32, tag="av",
                                       name="avA")
                    nc.vector.tensor_copy(out=avA, in_=avpA)
                    avB = av_pool.tile([D_HEAD + 1, QQ], F32, tag="av",
                                       name="avB")
                    nc.vector.tensor_copy(out=avB, in_=avpB)
                    pending_groups.append([
                        norm_start(avA, pair, 0, q0),
                        norm_start(avB, pair, 64, q0),
                    ])
                    if len(pending_groups) >= 2:
                        for st in pending_groups[-2]:
                            norm_step1(st)
                    depth = 3 if not last else 2
                    while len(pending_groups) >= depth:
                        for st in pending_groups.pop(0):
                            norm_step2(st)
                # drain any leftover proj chunks for the next pair
                for emit in proj_iter:
                    emit(pps)
            # ---- phase 3: tail blocks.  Same pool scope (yp reuses the
            # stp tag) so there is no pool barrier.  PSUM->SBUF copies
            # alternate DVE/ACT; stores split across the (now idle) sync +
            # gpsimd queues.  Blocks not gated by the final quarter's norms
            # run BEFORE the last norm-chain drain, so its inline recip/
            # gather waits overlap real PE work instead of stalling it.
            for tb in range(tail_emitted, 12):
                emit_tail_tb(tb, nc.vector if tb % 2 else nc.scalar,
                             store_q=nc.sync if tb % 2 else nc.gpsimd)
            for g in pending_groups:
                for st in g:
                    norm_step2(st)
            for tb in range(12, T // 128):
                emit_tail_tb(tb, nc.vector if tb % 2 else nc.scalar,
                             store_q=nc.sync if tb % 2 else nc.gpsimd,
                             split=tb >= 14)

    if split_waits:
        split_excess_waits(nc)
    return nc


_NC_CACHE = {}


def _get_nc(tkp):
    if tkp not in _NC_CACHE:
        _NC_CACHE[tkp] = build_nc(tkp)
    return _NC_CACHE[tkp]


def make_in_maps(x, mask, w_qkv, w_tail, tkp):
    """Shard full inputs into 8 per-core input maps (mask-compacted)."""
    import ml_dtypes
    bf16 = ml_dtypes.bfloat16
    x = np.asarray(x, dtype=np.float32)
    mask = np.asarray(mask, dtype=np.int32)
    w_qkv = np.asarray(w_qkv, dtype=np.float32)
    w_tail = np.asarray(w_tail, dtype=np.float32)

    w3 = w_qkv.reshape(N_HEAD, 3, D_HEAD, D_MODEL)  # [head, q|k|v, d, dmodel]
    in_maps = []
    for c in range(8):
        b, hg = c // 2, c % 2
        heads = list(range(hg * HPC, (hg + 1) * HPC))
        kept = np.nonzero(mask[b])[0]
        tk = len(kept)
        assert tk <= tkp
        # compacted x for k/v projections, zero-padded to tkp
        xk = np.zeros((tkp, D_MODEL), dtype=np.float32)
        xk[:tk] = x[b][kept]
        maskb = np.full((tkp,), NEG, dtype=np.float32)
        maskb[:tk] = 0.0
        # per-pair packed q/k weights: cols j*128+(0:64)=head 2j, (64:128)=2j+1
        wq = np.concatenate([w3[h, 0] for h in heads], axis=0)  # [512, 1024]
        wk = np.concatenate([w3[h, 1] for h in heads], axis=0)
        wv = np.concatenate([w3[h, 2] for h in heads], axis=0)
        wt = w_tail[:, hg * CAT:(hg + 1) * CAT]  # [1024, 512]
        in_maps.append({
            "ones8": np.ones((128, HPC), dtype=bf16),
            "xT": np.ascontiguousarray(x[b].T).astype(bf16),
            "xkT": np.ascontiguousarray(xk.T).astype(bf16),
            "wqT": np.ascontiguousarray(wq.T).astype(bf16),
            "wkT": np.ascontiguousarray(wk.T).astype(bf16),
            "wvT": np.ascontiguousarray(wv.T).astype(bf16),
            "wtailT": np.ascontiguousarray(wt.T).astype(bf16),
            "maskb": maskb,
        })
    return in_maps


def _tkp_for(mask):
    mask = np.asarray(mask)
    mx = max(int((mask[b] != 0).sum()) for b in range(mask.shape[0]))
    return max(128, ((mx + 127) // 128) * 128)


def kernel(x, mask, w_qkv, w_tail, b_tail):
    tkp = _tkp_for(mask)
    nc = _get_nc(tkp)
    in_maps = make_in_maps(x, mask, w_qkv, w_tail, tkp)
    last_err = None
    for _attempt in range(3):
        try:
            res = run_bass_kernel_spmd(nc, in_maps, list(range(8))).results
            break
        except Exception as e:  # transient device/runtime errors: retry
            last_err = e
            _time.sleep(3.0)
    else:
        raise last_err
    out = np.empty((BN, T, D_MODEL), dtype=np.float32)
    b_tail = np.asarray(b_tail, dtype=np.float32)
    for b in range(BN):
        out[b] = (np.asarray(res[2 * b]["y"], dtype=np.float32)
                  + np.asarray(res[2 * b + 1]["y"], dtype=np.float32)
                  + b_tail)
    return out



# revision 53
# speedup vs baseline: 1.0096x; 1.0096x over previous
"""Multi-head attention Trainium2 kernel, 8-way sharded, mask-compacted.

Problem: x[4,2048,1024] -> qkv proj (w_qkv [3072,1024]) -> 16-head attention
with key-padding mask -> tail proj (w_tail [1024,1024]) + b_tail.

Sharding: 8 shards = 4 batches x 2 head-groups (8 heads each). Each core
computes, for its (batch b, head-group hg):
  - q projection of x[b] for its 8 heads (all 2048 tokens)
  - k/v projections of the MASK-KEPT tokens only (host-compacted; masked
    keys contribute exactly zero to the softmax, so dropping them up front
    is exact and halves the attention work)
  - [kept x 2048] masked attention per head
  - partial tail matmul y_part = attn_cat @ w_tail[:, cat_slice].T
Host unshards: out[b] = y_part[2b] + y_part[2b+1] + b_tail.  No collectives.

All matmul operands are bf16 (PE 1 cyc/row); PSUM accumulation is f32.

Attention runs in (pair, 512-q-column-quarter) units: per key block both
heads' K=64 score matmuls write different column halves (different PSUM
banks) of one [128,1024] stp tile at row tiles (0,0)/(64,0), so they share
one WAR dep (the single exp that reads the whole tile) and execute
CONCURRENTLY in the PE array.  One ACT exp per key block covers both heads.
Next-pair k/q projection chunks and (in the last pair) ready tail blocks
fill the PE between score/AV groups.

Softmax denominator comes from a ones-column appended to V.  The per-token
reciprocal chain (DMA-scatter to 16 lanes -> DVE recip -> gather to
partitions 0/32 -> DVE stream_shuffle broadcast -> gpsimd multiply) is
software-pipelined two units deep so no engine FIFO ever blocks on a DMA;
a plain broadcast DMA explodes into 64x4KB packets and drowns the queue.
PE warm-up matmuls on memset scratch hold the HAM clock gate at K=8/8
through the DMA preamble.
"""

import time as _time

import numpy as np
from contextlib import ExitStack

import concourse.bass as bass
import concourse.mybir as mybir
import concourse.tile as tile
from concourse.bass_utils import run_bass_kernel_spmd

# ---------------------------------------------------------------------------
# walrus in this env accepts at most 2 sync waits per instruction; Tile's
# scheduler emits up to 10. Post-pass: peel excess waits onto same-engine
# NoOps inserted immediately before the offending instruction (same engine
# stream position => identical synchronization semantics).
MAX_WAITS = 1


def split_excess_waits(nc):
    for fn in nc.m.functions:
        for bb in fn.blocks:
            insts = list(bb.instructions)
            out = []
            changed = False
            for inst in insts:
                si = inst.sync_info
                waits = list(si.on_wait) if si is not None else []
                if len(waits) > MAX_WAITS:
                    extra = waits[:-MAX_WAITS]
                    for ci in range(0, len(extra), MAX_WAITS):
                        chunk = extra[ci:ci + MAX_WAITS]
                        nop = mybir.InstNoOp(
                            name=f"{inst.name}-ws{ci}", ins=[], outs=[])
                        nop.engine = inst.engine
                        nop.sync_info = mybir.SyncInfo(
                            on_wait=chunk, on_update=[])
                        out.append(nop)
                    inst.sync_info = mybir.SyncInfo(
                        on_wait=waits[-MAX_WAITS:],
                        on_update=list(si.on_update))
                    changed = True
                out.append(inst)
            if changed:
                bb.instructions = out
# ---------------------------------------------------------------------------

D_MODEL = 1024
N_HEAD = 16
D_HEAD = 64
BN, T = 4, 2048
HPC = 8                      # heads per core
NPAIR = HPC // 2             # head pairs (2 heads share a 128-row tile)
CAT = HPC * D_HEAD           # 512 per-core tail contraction
QH = T // 2                  # q processed in two halves of 1024
KC = D_MODEL // 128          # 8 contraction chunks
F32 = mybir.dt.float32
BF16 = mybir.dt.bfloat16
LAG = 4                      # ST->AV software pipeline depth (key blocks)
NEG = -30000.0               # additive bias for padded keys: exp -> 0


def build_nc(tkp, split_waits=True):
    """tkp: padded kept-key count (multiple of 128)."""
    nkbk = tkp // 128        # kept-key blocks
    nc = bass.Bass()
    xT = nc.declare_dram_parameter("xT", [D_MODEL, T], BF16, isOutput=False)
    xkT = nc.declare_dram_parameter("xkT", [D_MODEL, tkp], BF16, isOutput=False)
    wqT = nc.declare_dram_parameter("wqT", [D_MODEL, CAT], BF16, isOutput=False)
    wkT = nc.declare_dram_parameter("wkT", [D_MODEL, CAT], BF16, isOutput=False)
    wvT = nc.declare_dram_parameter("wvT", [D_MODEL, CAT], BF16, isOutput=False)
    wtailT = nc.declare_dram_parameter("wtailT", [CAT, D_MODEL], BF16, isOutput=False)
    maskb_d = nc.declare_dram_parameter("maskb", [tkp], F32, isOutput=False)
    ones8 = nc.declare_dram_parameter("ones8", [128, HPC], BF16, isOutput=False)
    y = nc.declare_dram_parameter("y", [T, D_MODEL], BF16, isOutput=True)

    with ExitStack() as ctx:
        tc = ctx.enter_context(tile.TileContext(nc))

        # ---- persistent pools
        const = ctx.enter_context(tc.tile_pool(name="const", bufs=1))
        wpool = ctx.enter_context(tc.tile_pool(name="w", bufs=1))
        xpool = ctx.enter_context(tc.tile_pool(name="x", bufs=1))
        qk_pool = ctx.enter_context(tc.tile_pool(name="qk", bufs=1))
        vaug_pool = ctx.enter_context(tc.tile_pool(name="vaug", bufs=1))
        num_pool = ctx.enter_context(tc.tile_pool(name="num", bufs=1))

        onesb = const.tile([128, HPC], BF16)
        nc.scalar.dma_start(out=onesb, in_=ones8[:, :])
        maskb = const.tile([128, nkbk], F32)
        nc.scalar.dma_start(
            out=maskb, in_=maskb_d.rearrange("(j p) -> p j", p=128))
        warm = const.tile([1, 1], F32)
        nc.scalar.copy(out=warm, in_=maskb[0:1, 0:1])  # preload ACT table
        # scratch operands for PE warm-up matmuls during the DMA preamble;
        # memset (engine op, no DMA) so the warm-ups start at ~0.5us, well
        # before the DMA queues spin up (~7us)
        wuscr = const.tile([128, 512], BF16)
        nc.gpsimd.memset(wuscr, 0.0)
        wub = const.tile([128, HPC], BF16)
        nc.gpsimd.memset(wub, 0.0)

        # weights resident in SBUF.  All DMAs are split per 128-row chunk so
        # consumers unblock as soon as their chunk lands, and ordered by
        # first use: xk + wv/wk (V & K proj) first, wtail last.
        wq_sb = wpool.tile([128, KC, CAT], BF16)
        wk_sb = wpool.tile([128, KC, CAT], BF16)
        wv_sb = wpool.tile([128, KC, CAT], BF16)
        wt_sb = wpool.tile([128, CAT // 128, D_MODEL], BF16)
        xkts = [xpool.tile([128, tkp], BF16, tag=f"xk{kc}", name=f"xk{kc}")
                for kc in range(KC)]
        xts = [xpool.tile([128, T], BF16, tag=f"x{kc}", name=f"x{kc}")
               for kc in range(KC)]
        # sync queue: xk (V/K proj inputs), then x half 0, then wtail
        for kc in range(KC):
            nc.sync.dma_start(out=xkts[kc],
                              in_=xkT[kc * 128:(kc + 1) * 128, :])
        for kc in range(KC):
            nc.sync.dma_start(
                out=xts[kc][:, 0:QH],
                in_=xT[kc * 128:(kc + 1) * 128, 0:QH])
        nc.sync.dma_start(out=wt_sb, in_=wtailT.rearrange("(c p) o -> p c o", p=128))
        # gpsimd software-DGE queue: weights per chunk, then x half 1
        for kc in range(KC):
            nc.gpsimd.dma_start(out=wv_sb[:, kc, :],
                                in_=wvT[kc * 128:(kc + 1) * 128, :])
        for kc in range(KC):
            nc.gpsimd.dma_start(out=wk_sb[:, kc, :],
                                in_=wkT[kc * 128:(kc + 1) * 128, :])
        for kc in range(KC):
            nc.gpsimd.dma_start(out=wq_sb[:, kc, :],
                                in_=wqT[kc * 128:(kc + 1) * 128, :])
        for kc in range(KC):
            nc.gpsimd.dma_start(
                out=xts[kc][:, QH:T],
                in_=xT[kc * 128:(kc + 1) * 128, QH:T])

        # persistent intermediates
        qts = [qk_pool.tile([128, T], BF16, tag=f"qt{j}", name=f"qt{j}")
               for j in range(NPAIR)]
        kts = [qk_pool.tile([128, tkp], BF16, tag=f"kt{j}", name=f"kt{j}")
               for j in range(NPAIR)]
        vaugs = [vaug_pool.tile([128, HPC, D_HEAD + 1], BF16, tag=f"va{t}",
                                name=f"va{t}") for t in range(nkbk)]
        nums = [num_pool.tile([128, T], BF16, tag=f"nm{j}", name=f"nm{j}")
                for j in range(NPAIR)]

        # ---- phase 1a: V projection (kept tokens), vps has all 8 PSUM banks
        with tc.tile_pool(name="vps", bufs=1, space="PSUM") as vps:
            # PE warm-up: dense dummy matmuls while the input DMAs land, so
            # the HAM clock-gate releases (K=8/8) before real work starts.
            # Reuses the vp0 slot (WAW: V proj's first group serializes after,
            # which is the PE program order anyway).
            wups = vps.tile([128, CAT], F32, tag="vp0", name="wu")
            for _ in range(14):
                nc.tensor.matmul(wups[0:HPC, :], wub, wuscr,
                                 start=True, stop=True, skip_group_check=True)
            done = 0
            while done < nkbk:
                g = min(8, nkbk - done)
                vp = [vps.tile([128, CAT], F32, tag=f"vp{i}", name=f"vp{i}")
                      for i in range(g)]
                for kc in range(KC):
                    for i in range(g):
                        nc.tensor.matmul(
                            vp[i],
                            xkts[kc][:, (done + i) * 128:(done + i + 1) * 128],
                            wv_sb[:, kc, :],
                            start=(kc == 0), stop=(kc == KC - 1),
                        )
                for i in range(g):
                    va = vaugs[done + i]
                    nc.gpsimd.memset(va[:, :, D_HEAD:D_HEAD + 1], 1.0)
                    nc.scalar.copy(
                        out=va[:, :, 0:D_HEAD],
                        in_=vp[i].rearrange("p (h d) -> p h d", h=HPC),
                    )
                done += g

        # ---- k/q projection helpers (pps pool: [128,512] x2 = 2 PSUM banks)
        KCH = [(c * 512, min(512, tkp - c * 512)) for c in range((tkp + 511) // 512)]

        def _copy(eng, out, in_):
            if eng is nc.scalar:
                eng.copy(out=out, in_=in_)
            else:
                eng.tensor_copy(out=out, in_=in_)

        def emit_kproj_chunk(pps, j, c0, csz, eng):
            pp = pps.tile([128, QH], F32, tag="stp", name="pp")
            for kc in range(KC):
                nc.tensor.matmul(
                    pp[:, 0:csz],
                    wk_sb[:, kc, j * 128:(j + 1) * 128],
                    xkts[kc][:, c0:c0 + csz],
                    start=(kc == 0), stop=(kc == KC - 1),
                )
            _copy(eng, kts[j][:, c0:c0 + csz], pp[:, 0:csz])

        def emit_qproj_chunk(pps, j, c, eng):
            pp = pps.tile([128, QH], F32, tag="stp", name="pp")
            for kc in range(KC):
                nc.tensor.matmul(
                    pp[:, 0:512],
                    wq_sb[:, kc, j * 128:(j + 1) * 128],
                    xts[kc][:, c * 512:(c + 1) * 512],
                    start=(kc == 0), stop=(kc == KC - 1),
                )
            _copy(eng, qts[j][:, c * 512:(c + 1) * 512], pp[:, 0:512])

        def proj_chunks_for_pair(j, eng):
            # k first (attention consumes k of every block before q half 2)
            for c0, csz in KCH:
                yield lambda pps, j=j, c0=c0, csz=csz, eng=eng: \
                    emit_kproj_chunk(pps, j, c0, csz, eng)
            for c in range(T // 512):
                yield lambda pps, j=j, c=c, eng=eng: \
                    emit_qproj_chunk(pps, j, c, eng)

# ---- phase 2: attention.  Units are (pair, quarter-of-512-q-columns):
        # per key block BOTH heads' K=64 score matmuls write different
        # column halves (= different PSUM banks) of ONE [128,1024] stp tile
        # at row tiles (0,0)/(64,0).  They share a single WAR dependency
        # (the one exp that reads the whole tile), so the Tile scheduler
        # keeps them adjacent and they execute CONCURRENTLY in the array —
        # the score phase costs ~1x instead of 2x.  One exp per key block
        # covers both heads; ACT paces the loop and projection chunks for
        # the next pair fill the PE slack.
        QQ = 512                 # q-columns per unit
        with tc.tile_pool(name="p_sb", bufs=4) as p_pool, \
             tc.tile_pool(name="r_sb", bufs=4) as r_pool, \
             tc.tile_pool(name="bc_sb", bufs=3) as bc_pool, \
             tc.tile_pool(name="av_sb", bufs=7) as av_pool, \
             tc.tile_pool(name="y_sb", bufs=3) as y_pool, \
             tc.tile_pool(name="stps", bufs=3, space="PSUM") as stps, \
             tc.tile_pool(name="avps", bufs=2, space="PSUM") as avps:
            pps = stps

            def emit_tail_tb(tb, eng, store_q=None):
                yp = stps.tile([128, D_MODEL], F32, tag="stp", name="yp")
                # c outer: the stationary operand (nums[c] slice) is shared
                # by the two n-slab matmuls — one weight load per c
                for c in range(CAT // 128):
                    for n in range(D_MODEL // 512):
                        nc.tensor.matmul(
                            yp[:, n * 512:(n + 1) * 512],
                            nums[c][:, tb * 128:(tb + 1) * 128],
                            wt_sb[:, c, n * 512:(n + 1) * 512],
                            start=(c == 0), stop=(c == CAT // 128 - 1),
                        )
                y_sb = y_pool.tile([128, D_MODEL], BF16, tag="ys")
                _copy(eng, y_sb, yp)
                # bf16 store; mid-attention fills ride the gpsimd queue so
                # sync stays free for the norm-chain hops
                (store_q or nc.gpsimd).dma_start(
                    out=y[tb * 128:(tb + 1) * 128, :], in_=y_sb)

            # pair-0 k/q projection up front (ACT idle: use it for copies)
            for emit in proj_chunks_for_pair(0, nc.scalar):
                emit(pps)

            def fat_keeper(ptile):
                # ~0.2us dummy matmul: fills PE when no proj work remains
                # (pair 3) so the HAM clock-gate stays at full speed
                nc.tensor.matmul(
                    ptile[0:HPC, 0:512], onesb, nums[0][:, 0:512],
                    start=True, stop=True, skip_group_check=True)

            # Per-token reciprocal of the ones-row, staged so that no DVE
            # instruction ever sits in the (strict-FIFO) vector queue waiting
            # on a DMA.  The denominator row is DMA-scattered over 16 lanes
            # (16 small packets — a full 64-partition broadcast DMA explodes
            # into 64x4KB packets at ~2.3us latency each and drowns the
            # hardware queue), recip'd on DVE, gathered back to partitions
            # 0 and 32, then broadcast across partitions with a DVE
            # stream_shuffle (mask=0: every lane of each 32-group reads
            # lane 0).  The final multiply runs on the idle gpsimd engine.
            def norm_start(av_sb, pair, r0, q0):
                d16 = r_pool.tile([16, QQ // 16], F32, tag="d64",
                                  name="d16")
                nc.sync.dma_start(
                    out=d16, in_=av_sb[D_HEAD:D_HEAD + 1, :])
                return [av_sb, pair, r0, q0, d16, None, 0]

            def norm_step1(st):
                if st[6] >= 1:
                    return
                av_sb, pair, r0, q0, d16, _, _ = st
                r16 = r_pool.tile([16, QQ // 16], F32, tag="r64",
                                  name="r16")
                nc.vector.reciprocal(out=r16, in_=d16)
                r2 = bc_pool.tile([D_HEAD, QQ], F32, tag="r2", name="r2")
                nc.sync.dma_start(out=r2[0:1, :], in_=r16)
                nc.sync.dma_start(out=r2[32:33, :], in_=r16)
                st[5] = r2
                st[6] = 1

            def norm_step2(st):
                norm_step1(st)
                av_sb, pair, r0, q0, d16, r2, _ = st
                bc_sb = bc_pool.tile([D_HEAD, QQ], F32, tag="bc",
                                     name="bc_sb")
                nc.vector.stream_shuffle(out=bc_sb, in_=r2, mask=[0] * 32)
                nc.gpsimd.tensor_tensor(
                    nums[pair][r0:r0 + 64, q0:q0 + QQ],
                    av_sb[0:D_HEAD, :], bc_sb,
                    mybir.AluOpType.mult,
                )

            LAG2 = 2  # ST->exp->AV pipeline depth (key blocks)
            pending_groups = []
            tail_emitted = 0
            for pair in range(NPAIR):
                proj_iter = iter(
                    proj_chunks_for_pair(pair + 1, nc.vector)
                    if pair + 1 < NPAIR else [])
                hA, hB = 2 * pair, 2 * pair + 1
                ktA = kts[pair][0:64, :]
                ktB = kts[pair][64:128, :]
                qtA = qts[pair][0:64, :]
                qtB = qts[pair][64:128, :]
                last = pair == NPAIR - 1
                for qtr in range(T // QQ):
                    q0 = qtr * QQ
                    avpA = avps.tile([D_HEAD + 1, QQ], F32, tag="avp",
                                     name="avpA")
                    avpB = avps.tile([D_HEAD + 1, QQ], F32, tag="avp",
                                     name="avpB")
                    p_tiles = {}

                    def emit_st_exp(kb):
                        # both heads' scores into different column halves
                        # (= different banks) of one stp tile: ST_A rows
                        # 0-63 / cols 0-511, ST_B rows 64-127 / cols
                        # 512-1023 — adjacent row-tiled matmuls, overlap
                        stp = stps.tile([128, 2 * QQ], F32, tag="stp",
                                        name="stp")
                        nc.tensor.matmul(
                            stp[:, 0:QQ],
                            ktA[:, kb * 128:(kb + 1) * 128],
                            qtA[:, q0:q0 + QQ],
                            start=True, stop=True,
                        )
                        nc.tensor.matmul(
                            stp[:, QQ:2 * QQ],
                            ktB[:, kb * 128:(kb + 1) * 128],
                            qtB[:, q0:q0 + QQ],
                            start=True, stop=True,
                        )
                        # one exp covers both heads' halves
                        p_sb = p_pool.tile([128, 2 * QQ], BF16, tag="p",
                                           name="p_sb")
                        nc.scalar.activation(
                            out=p_sb, in_=stp,
                            func=mybir.ActivationFunctionType.Exp,
                            bias=maskb[:, kb:kb + 1], scale=0.125,
                        )
                        p_tiles[kb] = p_sb

                    def emit_av(kb):
                        p_sb = p_tiles.pop(kb)
                        nc.tensor.matmul(
                            avpA, vaugs[kb][:, hA, :], p_sb[:, 0:QQ],
                            start=(kb == 0), stop=(kb == nkbk - 1),
                        )
                        nc.tensor.matmul(
                            avpB, vaugs[kb][:, hB, :], p_sb[:, QQ:2 * QQ],
                            start=(kb == 0), stop=(kb == nkbk - 1),
                        )

                    # filler budget: next-pair proj chunks (7 over 4 units);
                    # in the last pair, tail blocks whose q-columns are
                    # already fully normalized serve as fill instead
                    budget = (2 if qtr < 3 else 1) if not last else 3
                    for kb in range(nkbk):
                        if kb % 3 == 2 and budget > 0:
                            emit = next(proj_iter, None)
                            if emit is not None:
                                emit(pps)
                            elif last and qtr >= 2 and tail_emitted < 4 * (qtr - 1):
                                emit_tail_tb(tail_emitted, nc.vector)
                                tail_emitted += 1
                            budget -= 1
                        emit_st_exp(kb)
                        if kb >= LAG2:
                            emit_av(kb - LAG2)
                    for kb in range(max(0, nkbk - LAG2), nkbk):
                        emit_av(kb)
                    while budget > 0:
                        emit = next(proj_iter, None)
                        if emit is None:
                            break
                        emit(pps)
                        budget -= 1
                    # Evacuate both PSUM accumulators first (so the next
                    # unit's AVs never wait), then advance the norm chains:
                    # start(u) -> recip(u-1) -> multiply(u-2 mid / u-1 last
                    # pair); each stage's DMA inputs landed a unit ago, so
                    # the DVE FIFO never blocks on them.
                    avA = av_pool.tile([D_HEAD + 1, QQ], F32, tag="av",
                                       name="avA")
                    nc.vector.tensor_copy(out=avA, in_=avpA)
                    avB = av_pool.tile([D_HEAD + 1, QQ], F32, tag="av",
                                       name="avB")
                    nc.vector.tensor_copy(out=avB, in_=avpB)
                    pending_groups.append([
                        norm_start(avA, pair, 0, q0),
                        norm_start(avB, pair, 64, q0),
                    ])
                    if len(pending_groups) >= 2:
                        for st in pending_groups[-2]:
                            norm_step1(st)
                    depth = 3 if not last else 2
                    while len(pending_groups) >= depth:
                        for st in pending_groups.pop(0):
                            norm_step2(st)
                # drain any leftover proj chunks for the next pair
                for emit in proj_iter:
                    emit(pps)
            # ---- phase 3: tail blocks.  Same pool scope (yp reuses the
            # stp tag) so there is no pool barrier.  PSUM->SBUF copies
            # alternate DVE/ACT; stores split across the (now idle) sync +
            # gpsimd queues.  Blocks not gated by the final quarter's norms
            # run BEFORE the last norm-chain drain, so its inline recip/
            # gather waits overlap real PE work instead of stalling it.
            for tb in range(tail_emitted, 12):
                emit_tail_tb(tb, nc.vector if tb % 2 else nc.scalar,
                             store_q=nc.sync if tb % 2 else nc.gpsimd)
            for g in pending_groups:
                for st in g:
                    norm_step2(st)
            for tb in range(12, T // 128):
                emit_tail_tb(tb, nc.vector if tb % 2 else nc.scalar,
                             store_q=nc.sync if tb % 2 else nc.gpsimd)

    if split_waits:
        split_excess_waits(nc)
    return nc


_NC_CACHE = {}


def _get_nc(tkp):
    if tkp not in _NC_CACHE:
        _NC_CACHE[tkp] = build_nc(tkp)
    return _NC_CACHE[tkp]


def make_in_maps(x, mask, w_qkv, w_tail, tkp):
    """Shard full inputs into 8 per-core input maps (mask-compacted)."""
    import ml_dtypes
    bf16 = ml_dtypes.bfloat16
    x = np.asarray(x, dtype=np.float32)
    mask = np.asarray(mask, dtype=np.int32)
    w_qkv = np.asarray(w_qkv, dtype=np.float32)
    w_tail = np.asarray(w_tail, dtype=np.float32)

    w3 = w_qkv.reshape(N_HEAD, 3, D_HEAD, D_MODEL)  # [head, q|k|v, d, dmodel]
    in_maps = []
    for c in range(8):
        b, hg = c // 2, c % 2
        heads = list(range(hg * HPC, (hg + 1) * HPC))
        kept = np.nonzero(mask[b])[0]
        tk = len(kept)
        assert tk <= tkp
        # compacted x for k/v projections, zero-padded to tkp
        xk = np.zeros((tkp, D_MODEL), dtype=np.float32)
        xk[:tk] = x[b][kept]
        maskb = np.full((tkp,), NEG, dtype=np.float32)
        maskb[:tk] = 0.0
        # per-pair packed q/k weights: cols j*128+(0:64)=head 2j, (64:128)=2j+1
        wq = np.concatenate([w3[h, 0] for h in heads], axis=0)  # [512, 1024]
        wk = np.concatenate([w3[h, 1] for h in heads], axis=0)
        wv = np.concatenate([w3[h, 2] for h in heads], axis=0)
        wt = w_tail[:, hg * CAT:(hg + 1) * CAT]  # [1024, 512]
        in_maps.append({
            "ones8": np.ones((128, HPC), dtype=bf16),
            "xT": np.ascontiguousarray(x[b].T).astype(bf16),
            "xkT": np.ascontiguousarray(xk.T).astype(bf16),
            "wqT": np.ascontiguousarray(wq.T).astype(bf16),
            "wkT": np.ascontiguousarray(wk.T).astype(bf16),
            "wvT": np.ascontiguousarray(wv.T).astype(bf16),
            "wtailT": np.ascontiguousarray(wt.T).astype(bf16),
            "maskb": maskb,
        })
    return in_maps


def _tkp_for(mask):
    mask = np.asarray(mask)
    mx = max(int((mask[b] != 0).sum()) for b in range(mask.shape[0]))
    return max(128, ((mx + 127) // 128) * 128)


def kernel(x, mask, w_qkv, w_tail, b_tail):
    tkp = _tkp_for(mask)
    nc = _get_nc(tkp)
    in_maps = make_in_maps(x, mask, w_qkv, w_tail, tkp)
    last_err = None
    for _attempt in range(3):
        try:
            res = run_bass_kernel_spmd(nc, in_maps, list(range(8))).results
            break
        except Exception as e:  # transient device/runtime errors: retry
            last_err = e
            _time.sleep(3.0)
    else:
        raise last_err
    out = np.empty((BN, T, D_MODEL), dtype=np.float32)
    b_tail = np.asarray(b_tail, dtype=np.float32)
    for b in range(BN):
        out[b] = (np.asarray(res[2 * b]["y"], dtype=np.float32)
                  + np.asarray(res[2 * b + 1]["y"], dtype=np.float32)
                  + b_tail)
    return out

